# revision 6
# baseline (speedup 1.0000x reference)
"""Trainium2 kernel: binary-vector KNN min-L1-distance.

out[b] = min_r sum_d |states[b,d] - R[r,d]|,  states/R in {0,1}.

For binary values |s-r| = s + r - 2*s*r, so

    D[b,r] = sum_d states[b,d] + sum_d R[r,d]*(1 - 2*states[b,d])
           = S1[b] + (W @ R^T)[b,r],   W = 1 - 2*states  (+-1 valued)

which maps the O(B*R*D) distance computation onto the TensorEngine as a
single matmul, followed by a min-reduction over r on the VectorEngine.
All values are small integers, exact in bf16 with fp32 PSUM accumulation,
so the result is bit-exact.

Sharding: data-parallel over the batch axis, 1024 rows of `states` per
core, R replicated; no cross-core communication.

Host-side work is layout only: the transposes (TensorE needs the
contraction dim on partitions), the +-1 recode/bf16 cast, and the O(B*D)
row-sum S1 that the device adds back in the epilogue.
"""

import numpy as np
import ml_dtypes

import concourse.bass as bass
import concourse.mybir as mybir
import concourse.tile as tile
from concourse import bacc
from concourse.bass_utils import run_bass_kernel_spmd

B = 8192
NUM_REFS = 2048
DIM = 256
N_CORES = 8
B_LOC = B // N_CORES          # 1024 batch rows per core
BT = B_LOC // 128             # 8 batch tiles of 128 partitions
KT = DIM // 128               # 2 contraction tiles
RC = NUM_REFS // 512          # 4 reference chunks of 512 (one PSUM bank each)

BF16 = mybir.dt.bfloat16
F32 = mybir.dt.float32

_NC = None
LAST_RESULT = None


def _build():
    nc = bacc.Bacc()

    wT = nc.declare_dram_parameter("wT", [DIM, B_LOC], BF16, isOutput=False)
    rT = nc.declare_dram_parameter("rT", [DIM, NUM_REFS], BF16, isOutput=False)
    s1 = nc.declare_dram_parameter("s1", [128, BT], F32, isOutput=False)
    out = nc.declare_dram_parameter("out", [128, BT], F32, isOutput=True)

    with tile.TileContext(nc) as tc:
        with (
            tc.tile_pool(name="const", bufs=1) as const,
            tc.tile_pool(name="psum", bufs=2, space="PSUM") as psum_pool,
        ):
            # persistent SBUF tensors
            wt_sb = const.tile([128, KT * B_LOC], BF16)       # [p=d%128, k*1024+b]
            rt_sb = const.tile([128, KT * NUM_REFS], BF16)    # [p=d%128, k*2048+r]
            s1_sb = const.tile([128, BT], F32)
            mins = const.tile([128, BT], F32)
            out_sb = const.tile([128, BT], F32)

            # loads: split per k-tile (and per r-chunk for rT) so compute can
            # start as soon as the first pieces land
            for k in range(KT):
                nc.sync.dma_start(
                    wt_sb[:, k * B_LOC:(k + 1) * B_LOC],
                    wT[k * 128:(k + 1) * 128, :],
                )
            for k in range(KT):
                for rc in range(RC):
                    nc.sync.dma_start(
                        rt_sb[:, k * NUM_REFS + rc * 512:k * NUM_REFS + (rc + 1) * 512],
                        rT[k * 128:(k + 1) * 128, rc * 512:(rc + 1) * 512],
                    )
            nc.sync.dma_start(s1_sb[:], s1[:, :])

            for bt in range(BT):
                # 4 PSUM banks = the full [128, 2048] distance row-block
                ps = psum_pool.tile([128, NUM_REFS], F32)
                for k in range(KT):
                    lhsT = wt_sb[:, k * B_LOC + bt * 128:k * B_LOC + (bt + 1) * 128]
                    for rc in range(RC):
                        nc.tensor.matmul(
                            ps[:, rc * 512:(rc + 1) * 512],
                            lhsT,
                            rt_sb[:, k * NUM_REFS + rc * 512:k * NUM_REFS + (rc + 1) * 512],
                            start=(k == 0),
                            stop=(k == KT - 1),
                            skip_group_check=True,
                        )
                # min-reduce the whole [128, 2048] distance block -> [128, 1]
                # (DVE may read at most one non-scalar operand from PSUM, so a
                # single-input reduce is the right shape here)
                nc.vector.tensor_reduce(
                    mins[:, bt:bt + 1],
                    ps[:],
                    axis=mybir.AxisListType.X,
                    op=mybir.AluOpType.min,
                )

            # epilogue: add the states row-sum back in, store
            nc.vector.tensor_add(out_sb[:], mins[:], s1_sb[:])
            nc.sync.dma_start(out[:, :], out_sb[:])

    nc.compile()
    return nc


def _get_nc():
    global _NC
    if _NC is None:
        _NC = _build()
    return _NC


def kernel(states: np.ndarray, R: np.ndarray) -> np.ndarray:
    global LAST_RESULT
    states = np.asarray(states, dtype=np.float32)
    R = np.asarray(R, dtype=np.float32)

    W = (1.0 - 2.0 * states).astype(ml_dtypes.bfloat16)      # [B, DIM], +-1
    s1 = states.sum(axis=1, dtype=np.float32)                # [B]
    rT = np.ascontiguousarray(R.T.astype(ml_dtypes.bfloat16))  # [DIM, NUM_REFS]

    in_maps = []
    for c in range(N_CORES):
        sl = slice(c * B_LOC, (c + 1) * B_LOC)
        in_maps.append({
            "wT": np.ascontiguousarray(W[sl].T),                      # [DIM, B_LOC]
            "rT": rT,
            "s1": np.ascontiguousarray(s1[sl].reshape(BT, 128).T),    # [128, BT]
        })

    res = run_bass_kernel_spmd(_get_nc(), in_maps, core_ids=list(range(N_CORES)))
    LAST_RESULT = res

    full = np.empty(B, dtype=np.float32)
    for c in range(N_CORES):
        o = np.asarray(res.results[c]["out"])                 # [128, BT]
        full[c * B_LOC:(c + 1) * B_LOC] = o.T.reshape(-1)
    return full
